# revision 39
# baseline (speedup 1.0000x reference)
"""Trainium2 Bass kernel for nn_Attention_19739669692939 (sparse_attention).

Reference computation (shapes: L=1024, B=64, C=1024, D=512, E=512):
    Wa_e = W_attn[:, :C]        # [E, C]
    Wa_s = W_attn[:, C:]        # [E, D]
    pre  = enc_output @ Wa_e.T + s @ Wa_s.T     # [L, B, E] (s broadcast over L)
    engry = tanh(pre)
    att[b, l] = engry[l, b, :] @ W_v[0, :]
    out = softmax(att, axis=-1)                 # [B, 1024]

Distribution: pure data-parallel over batch. Core i handles batches
[8i, 8i+8); no collectives.

Per core: a 8192x1024 @ 1024x512 matmul in bf16 on the PE, fused bias+tanh
on ACT, and a masked-weight matmul reducing against W_v directly into
per-batch PSUM rows, then a free-axis softmax.

The PE contracts over partitions, so enc needs its feature dim (c) on
partitions: enc is cast f32->bf16 during the HBM load (free, SWDGE cast
path), then transposed on the PE ([128,128] is_transpose matmuls against
identity; bf16 halves the weight-load cost vs f32). f32->bf16 keeps rel
err ~2e-3, well under the 2e-2 gate.

Schedule notes (measured on HW):
- The main matmul (512 N=512 bf16 MMs) runs at the stream-rate roofline
  (~215 ns each, ~110 us/core = the bf16 compute roofline for 8.6 GFLOP).
- The per-batch W_v reductions are col-packed via tile_position so the
  four e-blocks execute concurrently in different 32-column PE groups.
- SWDGE ring order (s, W, first enc chunks) is chosen so the PE's data
  dependencies resolve in issue order during the ramp.
- Softmax skips the max-subtraction (logits bounded by ||W_v||_1 ~ 18)
  and fuses the row-sum into the exp via ACT's accum_out.
"""

import numpy as np

import concourse.bass as bass
import concourse.mybir as mybir
from concourse import bacc
from concourse.bass_utils import run_bass_kernel_spmd
from concourse.masks import make_identity
from concourse.tile import TileContext

F32 = mybir.dt.float32
BF16 = mybir.dt.bfloat16
AF = mybir.ActivationFunctionType

L = 1024          # enc length
B = 64            # global batch
BL = 8            # batch per core
C = 1024          # enc feature dim (2*enc_hid)
D = 512           # dec feature dim
E = 512           # engry dim
NCORES = 8

NCB = C // 128    # 8 c-blocks
NDB = D // 128    # 4 d-blocks
NEB = E // 128    # 4 e-blocks
LCH = 512         # l-chunk processed per inner iteration
NLC = L // LCH    # 2 chunks
KSUB = LCH // 128  # 4 l-subblocks per chunk

NWB = (C + D) // 128  # 12 blocks over W_attn's column (c/d) axis


def build_nc():
    nc = bacc.Bacc("TRN2", target_bir_lowering=False, debug=False)

    enc = nc.dram_tensor("enc_output", [L, BL, C], F32, kind="ExternalInput").ap()
    s = nc.dram_tensor("s", [1, BL, D], F32, kind="ExternalInput").ap()
    w_attn = nc.dram_tensor("W_attn", [E, C + D], F32, kind="ExternalInput").ap()
    w_v = nc.dram_tensor("W_v", [1, E], F32, kind="ExternalInput").ap()
    out = nc.dram_tensor("out", [BL, L], F32, kind="ExternalOutput").ap()

    with TileContext(nc) as tc:
        with (
            tc.tile_pool(name="consts", bufs=1) as consts,
            tc.tile_pool(name="nat", bufs=8) as nat_pool,
            tc.tile_pool(name="encT", bufs=4) as encT_pool,
            tc.tile_pool(name="engry", bufs=2) as engry_pool,
            tc.tile_pool(name="tp", bufs=4, space="PSUM") as tp_pool,
            tc.tile_pool(name="pre", bufs=2, space="PSUM") as pre_pool,
            tc.tile_pool(name="att", bufs=2, space="PSUM") as att_pool,
        ):
            # ---------------- setup: constants and weights ----------------
            ident = consts.tile([128, 128], F32, tag="ident")
            make_identity(nc, ident)
            identB = consts.tile([128, 128], BF16, tag="identB")
            nc.vector.tensor_copy(identB[:], ident[:])

            # s first on the SWDGE ring (tiny), then W, then the first enc
            # chunks — the ring drains in order, so order = priority.
            s_sbuf = consts.tile([BL, D], BF16, tag="s_sbuf")
            nc.gpsimd.dma_start(out=s_sbuf[:], in_=s[0])

            # SWDGE ring order = completion order: the first enc chunk goes
            # before W so the PE's first transposes have data ASAP; W's four
            # chunks next (waT transposes start incrementally); then more enc.
            NPRE = 4
            pre_enc = {}

            def issue_enc_cast(j):
                lc0, b0 = divmod(j, BL)
                enc_t = nat_pool.tile([128, KSUB * C], BF16, tag="nat",
                                      name=f"enc_pre{j}")
                nc.gpsimd.dma_start(
                    out=enc_t.rearrange("p (k w c) -> p k w c", k=KSUB, w=NCB),
                    in_=enc[lc0 * LCH:lc0 * LCH + LCH, b0, :].rearrange(
                        "(k p) (w c) -> p k w c", p=128, w=NCB
                    ),
                )
                pre_enc[(lc0, b0)] = enc_t

            # W_attn: cast-load bf16 [e'(128 part), (r 4, w 12, cc 128)],
            # split per e-block so waT transposes can start early.
            wnat = consts.tile([128, NEB * (C + D)], BF16, tag="wnat")
            for r in range(NEB):
                nc.gpsimd.dma_start(
                    out=wnat[:, r * (C + D):(r + 1) * (C + D)].rearrange(
                        "p (w c) -> p w c", w=NWB),
                    in_=w_attn[r * 128:(r + 1) * 128, :].rearrange(
                        "p (w c) -> p w c", w=NWB),
                )

            for j in range(NPRE):
                issue_enc_cast(j)

            # Small HAM-warmup block: bridges the DMA-gated idle slots in
            # the setup phase so the clock-gate sees sustained activity.
            # (Never read; bacc has no instruction-level DCE.)
            warm_ps = tp_pool.tile([128, 512], BF16, tag="tp")
            for _ in range(56):
                nc.tensor.transpose(warm_ps[:, :128], identB[:], identB[:])

            # s -> sT [d(4x128 part), b(8)] — first PE work (s lands first)
            sT = consts.tile([128, NDB * BL], BF16, tag="sT")
            for db in range(NDB):
                tps = tp_pool.tile([128, 512], BF16, tag="tp")
                nc.tensor.transpose(
                    tps[:, :BL],
                    s_sbuf[:, db * 128:(db + 1) * 128],
                    identB[:BL, :BL],
                )
                nc.vector.tensor_copy(sT[:, db * BL:(db + 1) * BL], tps[:, :BL])

            # waT [cc(128 part), (w 12, e 512)] via PE transposes (bf16),
            # r-outer so each W chunk unlocks a dense 12-transpose burst —
            # the bursts chain into >3.4us of sustained PE work, releasing
            # the HAM clock-gate during setup instead of mid-main-loop.
            waT = consts.tile([128, NWB * E], BF16, tag="waT")
            for r in range(NEB):
                for w in range(NWB):
                    tpw = tp_pool.tile([128, 512], BF16, tag="tp",
                                       name=f"tpw_{r}_{w}")
                    nc.tensor.transpose(
                        tpw[:, :128],
                        wnat[:, r * (C + D) + w * 128: r * (C + D) + (w + 1) * 128],
                        identB[:],
                    )
                    dst = waT[:, w * E + r * 128: w * E + (r + 1) * 128]
                    if w % 2 == 0:
                        nc.vector.tensor_copy(dst, tpw[:, :128])
                    else:
                        nc.scalar.copy(dst, tpw[:, :128])

            # bias[e, b] = Wa_s @ s[b].T  — [e(4x128 part), b(8)] per e-block
            bias_sbuf = consts.tile([128, NEB * BL], F32, tag="bias")
            for eb in range(NEB):
                bps = tp_pool.tile([128, 512], F32, tag="tp")
                for db in range(NDB):
                    nc.tensor.matmul(
                        bps[:, :BL],
                        lhsT=waT[:, (NCB + db) * E + eb * 128:
                                 (NCB + db) * E + (eb + 1) * 128],
                        rhs=sT[:, db * BL:(db + 1) * BL],
                        start=(db == 0),
                        stop=(db == NDB - 1),
                    )
                nc.vector.tensor_copy(bias_sbuf[:, eb * BL:(eb + 1) * BL], bps[:, :BL])

            # W_v: [1, E] -> wvT [e(128 part), eb(4)] via f32 PE transposes.
            wv_sbuf = consts.tile([1, E], F32, tag="wv_sbuf")
            nc.sync.dma_start(out=wv_sbuf[:], in_=w_v[:])
            wvT = consts.tile([128, NEB], F32, tag="wvT")
            for eb in range(NEB):
                tpv = tp_pool.tile([128, 512], F32, tag="tp")
                nc.tensor.transpose(
                    tpv[:, :1],
                    wv_sbuf[:, eb * 128:(eb + 1) * 128],
                    ident[:1, :1],
                )
                nc.vector.tensor_copy(wvT[:, eb:eb + 1], tpv[:, :1])

            # Masked W_v weights: for each (eb, b) a [128, 8] tile whose
            # column b holds wvT[:, eb], zeros elsewhere. Lets the W_v
            # contraction land in PSUM row b for batch b.
            wv_maskF = consts.tile([128, NEB * BL * BL], F32, tag="wv_maskF")
            nc.vector.memset(wv_maskF[:], 0.0)
            for eb in range(NEB):
                for b in range(BL):
                    nc.vector.tensor_copy(
                        wv_maskF[:, eb * BL * BL + b * BL + b:
                                 eb * BL * BL + b * BL + b + 1],
                        wvT[:, eb:eb + 1],
                    )
            wv_mask = consts.tile([128, NEB * BL * BL], BF16, tag="wv_mask")
            nc.vector.tensor_copy(wv_mask[:], wv_maskF[:])

            # exp(att) halves + their per-chunk row sums (softmax without the
            # max-subtraction: |att| <= ||W_v||_1 ~ 18, exp stays finite in f32)
            att_e = consts.tile([BL, L], F32, tag="att_e")
            sm_lc = consts.tile([BL, NLC], F32, tag="sm_lc")

            # ---------------- main loop ----------------
            for lc in range(NLC):
                # the 4 per-eb W_v reductions are col-packed: eb's result
                # lands in PSUM partitions [32eb, 32eb+8), accumulated over b
                att_ps = att_pool.tile([128, LCH], F32, tag="att")
                for b in range(BL):
                    l0 = lc * LCH
                    # enc chunk, cast f32->bf16 during DMA.
                    # layout: [p(128 l'), (k 4, cb 8, cc 128)]
                    if (lc, b) in pre_enc:
                        enc_t = pre_enc[(lc, b)]
                    else:
                        enc_t = nat_pool.tile([128, KSUB * C], BF16, tag="nat")
                        nc.gpsimd.dma_start(
                            out=enc_t.rearrange("p (k w c) -> p k w c",
                                                k=KSUB, w=NCB),
                            in_=enc[l0:l0 + LCH, b, :].rearrange(
                                "(k p) (w c) -> p k w c", p=128, w=NCB
                            ),
                        )
                    # PE transposes: two c-blocks share one full-bank PSUM
                    # tile (8 transposes, then a single [128,1024] copy).
                    # encT layout: [cc(128 part), (cb 8, l 512=k*128+l')]
                    encT = encT_pool.tile([128, NCB * LCH], BF16, tag="encT")
                    for cp in range(NCB // 2):
                        tpt = tp_pool.tile([128, 1024], BF16, tag="tp")
                        for half in range(2):
                            cb = 2 * cp + half
                            for k in range(KSUB):
                                nc.tensor.transpose(
                                    tpt[:, half * 512 + k * 128:
                                        half * 512 + (k + 1) * 128],
                                    enc_t[:, k * C + cb * 128:
                                          k * C + (cb + 1) * 128],
                                    identB[:],
                                )
                        if cp < 3:
                            nc.vector.tensor_copy(
                                encT[:, 2 * cp * LCH:(2 * cp + 2) * LCH], tpt[:])
                        else:
                            nc.scalar.copy(
                                encT[:, 2 * cp * LCH:(2 * cp + 2) * LCH], tpt[:])

                    engries = []
                    for eb in range(NEB):
                        pre = pre_pool.tile([128, LCH], F32, tag="pre")
                        for cb in range(NCB):
                            nc.tensor.matmul(
                                pre[:],
                                lhsT=waT[:, cb * E + eb * 128:
                                         cb * E + (eb + 1) * 128],
                                rhs=encT[:, cb * LCH:(cb + 1) * LCH],
                                start=(cb == 0),
                                stop=(cb == NCB - 1),
                            )
                        engry = engry_pool.tile([128, LCH], BF16, tag=f"engry{eb}",
                                                name=f"engry{eb}_{lc}_{b}")
                        nc.scalar.activation(
                            engry[:], pre[:], AF.Tanh,
                            bias=bias_sbuf[:, eb * BL + b: eb * BL + b + 1],
                            scale=1.0,
                        )
                        engries.append(engry)
                    # back-to-back col-packed W_v reductions (concurrent in
                    # the PE's four 32-column groups)
                    for eb in range(NEB):
                        nc.tensor.matmul(
                            att_ps[32 * eb:32 * eb + BL, :],
                            lhsT=wv_mask[:, eb * BL * BL + b * BL:
                                         eb * BL * BL + (b + 1) * BL],
                            rhs=engries[eb][:],
                            start=(b == 0),
                            stop=(b == BL - 1),
                            tile_position=(0, 32 * eb),
                        )
                # sum the four col-group blocks (partition-offset reads off
                # PSUM), then exp with the row-sum accumulated in the same
                # ACT instruction.
                t01 = consts.tile([BL, LCH], F32, tag="t01",
                                  name=f"t01_{lc}")
                t23 = consts.tile([BL, LCH], F32, tag="t23",
                                  name=f"t23_{lc}")
                att_s = consts.tile([BL, LCH], F32, tag="att_s",
                                    name=f"att_s{lc}")
                nc.vector.tensor_copy(t01[:], att_ps[0:BL, :])
                nc.vector.tensor_add(t23[:], t01[:], att_ps[32:32 + BL, :])
                nc.vector.tensor_add(t01[:], t23[:], att_ps[64:64 + BL, :])
                nc.vector.tensor_add(att_s[:], t01[:], att_ps[96:96 + BL, :])
                nc.scalar.activation(
                    att_e[:, lc * LCH:(lc + 1) * LCH], att_s[:], AF.Exp,
                    accum_out=sm_lc[:, lc:lc + 1],
                )

            # ---------------- softmax epilogue ----------------
            sm = consts.tile([BL, 1], F32, tag="sm")
            rs = consts.tile([BL, 1], F32, tag="rs")
            att_o = consts.tile([BL, L], F32, tag="att_o")

            nc.vector.tensor_add(sm[:], sm_lc[:, 0:1], sm_lc[:, 1:2])
            nc.vector.reciprocal(rs[:], sm[:])
            nc.vector.tensor_scalar_mul(att_o[:], att_e[:], rs[:, 0:1])
            nc.sync.dma_start(out=out[:], in_=att_o[:])

    nc.compile()
    return nc


_NC_CACHE = None


def _get_nc():
    global _NC_CACHE
    if _NC_CACHE is None:
        _NC_CACHE = build_nc()
    return _NC_CACHE


def make_in_maps(enc_output, s, W_attn, W_v):
    enc_output = np.asarray(enc_output, dtype=np.float32)
    s = np.asarray(s, dtype=np.float32)
    W_attn = np.ascontiguousarray(np.asarray(W_attn, dtype=np.float32))
    W_v = np.ascontiguousarray(np.asarray(W_v, dtype=np.float32))
    in_maps = []
    for i in range(NCORES):
        in_maps.append({
            "enc_output": np.ascontiguousarray(enc_output[:, i * BL:(i + 1) * BL, :]),
            "s": np.ascontiguousarray(s[:, i * BL:(i + 1) * BL, :]),
            "W_attn": W_attn,
            "W_v": W_v,
        })
    return in_maps


def kernel(enc_output, s, W_attn, W_v):
    nc = _get_nc()
    in_maps = make_in_maps(enc_output, s, W_attn, W_v)
    res = run_bass_kernel_spmd(nc, in_maps, core_ids=list(range(NCORES)))
    return np.concatenate([res.results[i]["out"] for i in range(NCORES)], axis=0)


# revision 40
# speedup vs baseline: 1.0116x; 1.0116x over previous
"""Trainium2 Bass kernel for nn_Attention_19739669692939 (sparse_attention).

Reference computation (shapes: L=1024, B=64, C=1024, D=512, E=512):
    Wa_e = W_attn[:, :C]        # [E, C]
    Wa_s = W_attn[:, C:]        # [E, D]
    pre  = enc_output @ Wa_e.T + s @ Wa_s.T     # [L, B, E] (s broadcast over L)
    engry = tanh(pre)
    att[b, l] = engry[l, b, :] @ W_v[0, :]
    out = softmax(att, axis=-1)                 # [B, 1024]

Distribution: pure data-parallel over batch. Core i handles batches
[8i, 8i+8); no collectives.

Per core: a 8192x1024 @ 1024x512 matmul in bf16 on the PE, fused bias+tanh
on ACT, and a masked-weight matmul reducing against W_v directly into
per-batch PSUM rows, then a free-axis softmax.

The PE contracts over partitions, so enc needs its feature dim (c) on
partitions: enc is cast f32->bf16 during the HBM load (free, SWDGE cast
path), then transposed on the PE ([128,128] is_transpose matmuls against
identity; bf16 halves the weight-load cost vs f32). f32->bf16 keeps rel
err ~2e-3, well under the 2e-2 gate.

Schedule notes (measured on HW):
- The main matmul (512 N=512 bf16 MMs) runs at the stream-rate roofline
  (~215 ns each, ~110 us/core = the bf16 compute roofline for 8.6 GFLOP).
- The per-batch W_v reductions are col-packed via tile_position so the
  four e-blocks execute concurrently in different 32-column PE groups.
- SWDGE ring order (s, W, first enc chunks) is chosen so the PE's data
  dependencies resolve in issue order during the ramp.
- Softmax skips the max-subtraction (logits bounded by ||W_v||_1 ~ 18)
  and fuses the row-sum into the exp via ACT's accum_out.
"""

import numpy as np

import concourse.bass as bass
import concourse.mybir as mybir
from concourse import bacc
from concourse.bass_utils import run_bass_kernel_spmd
from concourse.masks import make_identity
from concourse.tile import TileContext

F32 = mybir.dt.float32
BF16 = mybir.dt.bfloat16
AF = mybir.ActivationFunctionType

L = 1024          # enc length
B = 64            # global batch
BL = 8            # batch per core
C = 1024          # enc feature dim (2*enc_hid)
D = 512           # dec feature dim
E = 512           # engry dim
NCORES = 8

NCB = C // 128    # 8 c-blocks
NDB = D // 128    # 4 d-blocks
NEB = E // 128    # 4 e-blocks
LCH = 512         # l-chunk processed per inner iteration
NLC = L // LCH    # 2 chunks
KSUB = LCH // 128  # 4 l-subblocks per chunk

NWB = (C + D) // 128  # 12 blocks over W_attn's column (c/d) axis


def build_nc():
    nc = bacc.Bacc("TRN2", target_bir_lowering=False, debug=False)

    enc = nc.dram_tensor("enc_output", [L, BL, C], F32, kind="ExternalInput").ap()
    s = nc.dram_tensor("s", [1, BL, D], F32, kind="ExternalInput").ap()
    w_attn = nc.dram_tensor("W_attn", [E, C + D], F32, kind="ExternalInput").ap()
    w_v = nc.dram_tensor("W_v", [1, E], F32, kind="ExternalInput").ap()
    out = nc.dram_tensor("out", [BL, L], F32, kind="ExternalOutput").ap()

    with TileContext(nc) as tc:
        with (
            tc.tile_pool(name="consts", bufs=1) as consts,
            tc.tile_pool(name="nat", bufs=8) as nat_pool,
            tc.tile_pool(name="encT", bufs=4) as encT_pool,
            tc.tile_pool(name="engry", bufs=2) as engry_pool,
            tc.tile_pool(name="tp", bufs=4, space="PSUM") as tp_pool,
            tc.tile_pool(name="pre", bufs=2, space="PSUM") as pre_pool,
            tc.tile_pool(name="att", bufs=2, space="PSUM") as att_pool,
        ):
            # ---------------- setup: constants and weights ----------------
            ident = consts.tile([128, 128], F32, tag="ident")
            make_identity(nc, ident)
            identB = consts.tile([128, 128], BF16, tag="identB")
            nc.vector.tensor_copy(identB[:], ident[:])

            # s first on the SWDGE ring (tiny), then W, then the first enc
            # chunks — the ring drains in order, so order = priority.
            s_sbuf = consts.tile([BL, D], BF16, tag="s_sbuf")
            nc.gpsimd.dma_start(out=s_sbuf[:], in_=s[0])

            # SWDGE ring order = completion order: the first enc chunk goes
            # before W so the PE's first transposes have data ASAP; W's four
            # chunks next (waT transposes start incrementally); then more enc.
            NPRE = 4
            pre_enc = {}

            def issue_enc_cast(j):
                lc0, b0 = divmod(j, BL)
                enc_t = nat_pool.tile([128, KSUB * C], BF16, tag="nat",
                                      name=f"enc_pre{j}")
                nc.gpsimd.dma_start(
                    out=enc_t.rearrange("p (k w c) -> p k w c", k=KSUB, w=NCB),
                    in_=enc[lc0 * LCH:lc0 * LCH + LCH, b0, :].rearrange(
                        "(k p) (w c) -> p k w c", p=128, w=NCB
                    ),
                )
                pre_enc[(lc0, b0)] = enc_t

            # W_attn: cast-load bf16 [e'(128 part), (r 4, w 12, cc 128)],
            # split per e-block so waT transposes can start early.
            wnat = consts.tile([128, NEB * (C + D)], BF16, tag="wnat")
            for r in range(NEB):
                nc.gpsimd.dma_start(
                    out=wnat[:, r * (C + D):(r + 1) * (C + D)].rearrange(
                        "p (w c) -> p w c", w=NWB),
                    in_=w_attn[r * 128:(r + 1) * 128, :].rearrange(
                        "p (w c) -> p w c", w=NWB),
                )

            for j in range(NPRE):
                issue_enc_cast(j)

            # Small HAM-warmup block: bridges the DMA-gated idle slots in
            # the setup phase so the clock-gate sees sustained activity.
            # (Never read; bacc has no instruction-level DCE.)
            warm_ps = tp_pool.tile([128, 512], BF16, tag="tp")
            for _ in range(56):
                nc.tensor.transpose(warm_ps[:, :128], identB[:], identB[:])

            # s -> sT [d(4x128 part), b(8)] — first PE work (s lands first)
            sT = consts.tile([128, NDB * BL], BF16, tag="sT")
            for db in range(NDB):
                tps = tp_pool.tile([128, 512], BF16, tag="tp")
                nc.tensor.transpose(
                    tps[:, :BL],
                    s_sbuf[:, db * 128:(db + 1) * 128],
                    identB[:BL, :BL],
                )
                nc.vector.tensor_copy(sT[:, db * BL:(db + 1) * BL], tps[:, :BL])

            # waT [cc(128 part), (w 12, e 512)] via PE transposes (bf16),
            # r-outer so each W chunk unlocks a dense 12-transpose burst —
            # the bursts chain into >3.4us of sustained PE work, releasing
            # the HAM clock-gate during setup instead of mid-main-loop.
            waT = consts.tile([128, NWB * E], BF16, tag="waT")
            for r in range(NEB):
                for w in range(NWB):
                    tpw = tp_pool.tile([128, 512], BF16, tag="tp",
                                       name=f"tpw_{r}_{w}")
                    nc.tensor.transpose(
                        tpw[:, :128],
                        wnat[:, r * (C + D) + w * 128: r * (C + D) + (w + 1) * 128],
                        identB[:],
                    )
                    dst = waT[:, w * E + r * 128: w * E + (r + 1) * 128]
                    if w % 2 == 0:
                        nc.vector.tensor_copy(dst, tpw[:, :128])
                    else:
                        nc.scalar.copy(dst, tpw[:, :128])

            # bias[e, b] = Wa_s @ s[b].T  — [e(4x128 part), b(8)] per e-block
            bias_sbuf = consts.tile([128, NEB * BL], F32, tag="bias")
            for eb in range(NEB):
                bps = tp_pool.tile([128, 512], F32, tag="tp")
                for db in range(NDB):
                    nc.tensor.matmul(
                        bps[:, :BL],
                        lhsT=waT[:, (NCB + db) * E + eb * 128:
                                 (NCB + db) * E + (eb + 1) * 128],
                        rhs=sT[:, db * BL:(db + 1) * BL],
                        start=(db == 0),
                        stop=(db == NDB - 1),
                    )
                nc.vector.tensor_copy(bias_sbuf[:, eb * BL:(eb + 1) * BL], bps[:, :BL])

            # W_v: [1, E] -> wvT [e(128 part), eb(4)] via f32 PE transposes.
            wv_sbuf = consts.tile([1, E], F32, tag="wv_sbuf")
            nc.sync.dma_start(out=wv_sbuf[:], in_=w_v[:])
            wvT = consts.tile([128, NEB], F32, tag="wvT")
            for eb in range(NEB):
                tpv = tp_pool.tile([128, 512], F32, tag="tp")
                nc.tensor.transpose(
                    tpv[:, :1],
                    wv_sbuf[:, eb * 128:(eb + 1) * 128],
                    ident[:1, :1],
                )
                nc.vector.tensor_copy(wvT[:, eb:eb + 1], tpv[:, :1])

            # Masked W_v weights: for each (eb, b) a [128, 8] tile whose
            # column b holds wvT[:, eb], zeros elsewhere. Lets the W_v
            # contraction land in PSUM row b for batch b.
            wv_maskF = consts.tile([128, NEB * BL * BL], F32, tag="wv_maskF")
            nc.vector.memset(wv_maskF[:], 0.0)
            for eb in range(NEB):
                for b in range(BL):
                    nc.vector.tensor_copy(
                        wv_maskF[:, eb * BL * BL + b * BL + b:
                                 eb * BL * BL + b * BL + b + 1],
                        wvT[:, eb:eb + 1],
                    )
            wv_mask = consts.tile([128, NEB * BL * BL], BF16, tag="wv_mask")
            nc.vector.tensor_copy(wv_mask[:], wv_maskF[:])

            # exp(att) halves + their per-chunk row sums (softmax without the
            # max-subtraction: |att| <= ||W_v||_1 ~ 18, exp stays finite in f32)
            att_e = consts.tile([BL, L], F32, tag="att_e")
            sm_lc = consts.tile([BL, NLC], F32, tag="sm_lc")

            # ---------------- main loop ----------------
            for lc in range(NLC):
                # the 4 per-eb W_v reductions are col-packed: eb's result
                # lands in PSUM partitions [32eb, 32eb+8), accumulated over b
                att_ps = att_pool.tile([128, LCH], F32, tag="att")
                for b in range(BL):
                    l0 = lc * LCH
                    # enc chunk, cast f32->bf16 during DMA.
                    # layout: [p(128 l'), (k 4, cb 8, cc 128)]
                    if (lc, b) in pre_enc:
                        enc_t = pre_enc[(lc, b)]
                    else:
                        enc_t = nat_pool.tile([128, KSUB * C], BF16, tag="nat")
                        nc.gpsimd.dma_start(
                            out=enc_t.rearrange("p (k w c) -> p k w c",
                                                k=KSUB, w=NCB),
                            in_=enc[l0:l0 + LCH, b, :].rearrange(
                                "(k p) (w c) -> p k w c", p=128, w=NCB
                            ),
                        )
                    # PE transposes: two c-blocks share one full-bank PSUM
                    # tile (8 transposes, then a single [128,1024] copy).
                    # encT layout: [cc(128 part), (cb 8, l 512=k*128+l')]
                    encT = encT_pool.tile([128, NCB * LCH], BF16, tag="encT")
                    for cp in range(NCB // 2):
                        tpt = tp_pool.tile([128, 1024], BF16, tag="tp")
                        for half in range(2):
                            cb = 2 * cp + half
                            for k in range(KSUB):
                                nc.tensor.transpose(
                                    tpt[:, half * 512 + k * 128:
                                        half * 512 + (k + 1) * 128],
                                    enc_t[:, k * C + cb * 128:
                                          k * C + (cb + 1) * 128],
                                    identB[:],
                                )
                        if cp < 3:
                            nc.vector.tensor_copy(
                                encT[:, 2 * cp * LCH:(2 * cp + 2) * LCH], tpt[:])
                        else:
                            nc.scalar.copy(
                                encT[:, 2 * cp * LCH:(2 * cp + 2) * LCH], tpt[:])

                    engries = []
                    for eb in range(NEB):
                        pre = pre_pool.tile([128, LCH], F32, tag="pre")
                        for cb in range(NCB):
                            nc.tensor.matmul(
                                pre[:],
                                lhsT=waT[:, cb * E + eb * 128:
                                         cb * E + (eb + 1) * 128],
                                rhs=encT[:, cb * LCH:(cb + 1) * LCH],
                                start=(cb == 0),
                                stop=(cb == NCB - 1),
                            )
                        engry = engry_pool.tile([128, LCH], BF16, tag=f"engry{eb}",
                                                name=f"engry{eb}_{lc}_{b}")
                        nc.scalar.activation(
                            engry[:], pre[:], AF.Tanh,
                            bias=bias_sbuf[:, eb * BL + b: eb * BL + b + 1],
                            scale=1.0,
                        )
                        engries.append(engry)
                    # back-to-back col-packed W_v reductions (concurrent in
                    # the PE's four 32-column groups)
                    for eb in range(NEB):
                        nc.tensor.matmul(
                            att_ps[32 * eb:32 * eb + BL, :],
                            lhsT=wv_mask[:, eb * BL * BL + b * BL:
                                         eb * BL * BL + (b + 1) * BL],
                            rhs=engries[eb][:],
                            start=(b == 0),
                            stop=(b == BL - 1),
                            tile_position=(0, 32 * eb),
                        )
                # sum the four col-group blocks (partition-offset reads off
                # PSUM), then exp with the row-sum accumulated in the same
                # ACT instruction.
                t01 = consts.tile([BL, LCH], F32, tag="t01",
                                  name=f"t01_{lc}")
                t23 = consts.tile([BL, LCH], F32, tag="t23",
                                  name=f"t23_{lc}")
                att_s = consts.tile([BL, LCH], F32, tag="att_s",
                                    name=f"att_s{lc}")
                nc.vector.tensor_copy(t01[:], att_ps[0:BL, :])
                nc.vector.tensor_add(t23[:], t01[:], att_ps[32:32 + BL, :])
                nc.vector.tensor_add(t01[:], t23[:], att_ps[64:64 + BL, :])
                nc.vector.tensor_add(att_s[:], t01[:], att_ps[96:96 + BL, :])
                nc.scalar.activation(
                    att_e[:, lc * LCH:(lc + 1) * LCH], att_s[:], AF.Exp,
                    accum_out=sm_lc[:, lc:lc + 1],
                )

            # ---------------- softmax epilogue ----------------
            sm = consts.tile([BL, 1], F32, tag="sm")
            rs = consts.tile([BL, 1], F32, tag="rs")
            att_o = consts.tile([BL, L], F32, tag="att_o")

            nc.vector.tensor_add(sm[:], sm_lc[:, 0:1], sm_lc[:, 1:2])
            nc.vector.reciprocal(rs[:], sm[:])
            # scale + store in halves: the second multiply overlaps the
            # first output DMA's wire time
            nc.vector.tensor_scalar_mul(att_o[:, :LCH], att_e[:, :LCH],
                                        rs[:, 0:1])
            nc.sync.dma_start(out=out[:, :LCH], in_=att_o[:, :LCH])
            nc.vector.tensor_scalar_mul(att_o[:, LCH:], att_e[:, LCH:],
                                        rs[:, 0:1])
            nc.sync.dma_start(out=out[:, LCH:], in_=att_o[:, LCH:])

    nc.compile()
    return nc


_NC_CACHE = None


def _get_nc():
    global _NC_CACHE
    if _NC_CACHE is None:
        _NC_CACHE = build_nc()
    return _NC_CACHE


def make_in_maps(enc_output, s, W_attn, W_v):
    enc_output = np.asarray(enc_output, dtype=np.float32)
    s = np.asarray(s, dtype=np.float32)
    W_attn = np.ascontiguousarray(np.asarray(W_attn, dtype=np.float32))
    W_v = np.ascontiguousarray(np.asarray(W_v, dtype=np.float32))
    in_maps = []
    for i in range(NCORES):
        in_maps.append({
            "enc_output": np.ascontiguousarray(enc_output[:, i * BL:(i + 1) * BL, :]),
            "s": np.ascontiguousarray(s[:, i * BL:(i + 1) * BL, :]),
            "W_attn": W_attn,
            "W_v": W_v,
        })
    return in_maps


def kernel(enc_output, s, W_attn, W_v):
    nc = _get_nc()
    in_maps = make_in_maps(enc_output, s, W_attn, W_v)
    res = run_bass_kernel_spmd(nc, in_maps, core_ids=list(range(NCORES)))
    return np.concatenate([res.results[i]["out"] for i in range(NCORES)], axis=0)
